# revision 20
# baseline (speedup 1.0000x reference)
"""Trainium2 Bass kernel for causal top-K cosine-similarity GNN message passing.

Module: delta = gelu(mix*x + (1-mix)*msg) * scale, where msg is the mean of
the K=8 causally-preceding neighbors with highest cosine similarity.

Strategy (8 NeuronCores, SPMD):
  - batch b -> core pair (2b, 2b+1). Per batch, 16 query units of 256 rows;
    even core takes units {15,13,...,1}, odd {14,12,...,0}. All cores run an
    identical program under the shared key-width schedule WS=[16,14,...,2]
    (x256 keys); causal masking is data-driven (additive -1e30 mask inputs),
    so padding columns self-mask.
  - resident keys: 16 R-positions of normalized-transposed x. Even cores hold
    units 0..15; odd cores hold [dup(unit0), units 0..14] (one-unit shift), so
    slot s's query unit sits at R-position WS[s]-1 for BOTH parities and the
    sim-matmul stationary (and the diagonal rhs) are SLICED from the resident
    buffer -- no separate query-side DMA stream. The dup/invalid first unit on
    odd cores is masked by an additive first-tile mask (cmf); the last-tile
    diagonal mask (cm) becomes parity-independent (f <= row + U).
  - sim tiles via PE matmul (fp32r operands: full speed, ~2.8e-6 rms noise);
    per-row 8th-largest via the DVE max8 instruction; binary adjacency
    A = (sim >= max(tau, -2)) selects exactly the top-8 (ties measure-zero).
  - msg = (A @ x)/8 as an fp8(e4m3) DoubleRow matmul (2x PE rate, half DMA
    bytes). A is exact in fp8 (0/1); x quantized to e4m3 adds ~8e-3 rel err
    (measured), well inside the 2e-2 gate. A tiles are PE-transposed to put
    the contraction dim on partitions; xk streamed as row-paired fp8 tiles.
  - epilogue: pmsg += (mix/msgc)*x via ACT-copy + DVE add, then a single
    ACT Gelu with input scale=msgc; output stored bf16, final *scale applied
    on the host (monotone rounding unaffected).
  - Rows 0..6 of each batch (fewer than 8 neighbors) are fixed up exactly on
    the host (28 of 16384 rows).
"""

import os
import sys

if "/opt/trn_rl_repo" not in sys.path:
    sys.path.insert(0, "/opt/trn_rl_repo")

_NOGELU = bool(os.environ.get("KERNEL_SIM_NOGELU"))  # CoreSim lacks Gelu

import numpy as np
import ml_dtypes

import concourse.bacc as bacc
import concourse.mybir as mybir
import concourse.tile as tile
from concourse.bass_utils import run_bass_kernel_spmd
from concourse.masks import make_identity

B, T, D, K = 4, 4096, 1024, 8
U = 256                      # unit (query block) size
WS = [16, 14, 12, 10, 8, 6, 4, 2]   # per-slot key width, in units
NSLOT = len(WS)
QPC = NSLOT * U              # query rows per core (2048)
NRES = 16                    # resident R-positions (even: units 0..15; odd: dup+0..14)
NEG = -1.0e30

f32 = mybir.dt.float32
f32r = mybir.dt.float32r
bf16 = mybir.dt.bfloat16
fp8 = mybir.dt.float8e4
AF = mybir.ActivationFunctionType
ALU = mybir.AluOpType
PM = mybir.MatmulPerfMode

SIM_DT = "f32r"              # "f32r" (fast, tf32-like) or "f32" (exact, 4x slower sim)

_PROG_CACHE = {}


def _build_program(sim_dt_key, trivial_affine, mix, scale):
    sdt = f32r if sim_dt_key == "f32r" else f32
    msgc = (1.0 - mix) / K
    # epilogue fuses mix*x into the PSUM accumulator pre-scaled by 1/msgc;
    # requires msgc not absurdly small (harness: mix=0.5 -> msgc=0.0625).
    fuse_ok = msgc > 1e-4

    nc = bacc.Bacc("TRN2", target_bir_lowering=False, debug=False)

    xnt_d = nc.dram_tensor("xnt", [D, NRES * U], sdt, kind="ExternalInput")
    xk8_d = nc.dram_tensor("xk8", [T // U, 128, 2 * D], fp8, kind="ExternalInput")
    # xq is host-prescaled by (mix/msgc) [fuse] or mix [no-fuse] so the
    # epilogue is a single DVE add into PSUM (no ACT copy).
    xq_d = nc.dram_tensor("xq", [QPC, D], bf16, kind="ExternalInput")
    cm_d = nc.dram_tensor("cmask", [2, 128, 2 * U], f32, kind="ExternalInput")
    cmf_d = nc.dram_tensor("cmaskf", [128, 2 * U], f32, kind="ExternalInput")
    if not trivial_affine:
        gain_d = nc.dram_tensor("gain", [1, D], f32, kind="ExternalInput")
        bias_d = nc.dram_tensor("bias", [1, D], f32, kind="ExternalInput")
    out_d = nc.dram_tensor("out", [QPC, D], bf16, kind="ExternalOutput")

    with tile.TileContext(nc) as tc:
        with (
            tc.tile_pool(name="res", bufs=1) as res_pool,
            tc.tile_pool(name="simp", bufs=3) as sim_pool,
            tc.tile_pool(name="xkp", bufs=5) as xk_pool,
            tc.tile_pool(name="atp", bufs=2) as at_pool,
            tc.tile_pool(name="atrp", bufs=4) as atr_pool,
            tc.tile_pool(name="m8p", bufs=2) as m8_pool,
            tc.tile_pool(name="xqep", bufs=2) as xqe_pool,
            tc.tile_pool(name="otp", bufs=2) as ot_pool,
            tc.tile_pool(name="ps_sim", bufs=3, space="PSUM") as psim_pool,
            tc.tile_pool(name="ps_tr", bufs=1, space="PSUM") as ptr_pool,
            tc.tile_pool(name="ps_msg", bufs=4, space="PSUM") as pmsg_pool,
        ):
            # ---- setup: resident keys (16 R-units) as 2-unit tiles. One 3D
            # DMA per tile (all 8 d-chunks), alternating gpsimd/scalar queues
            # in first-use order. The sync queue is reserved for the xk8
            # stream: a consumption-paced ring DMA ahead of resident chunks
            # would head-of-line-block them.
            cm_sb = res_pool.tile([128, 2 * 2 * U], f32, tag="cm")
            for h in range(2):
                nc.scalar.dma_start(out=cm_sb[:, h * 2 * U:(h + 1) * 2 * U], in_=cm_d[h])
            cmf_sb = res_pool.tile([128, 2 * U], f32, tag="cmf")
            nc.scalar.dma_start(out=cmf_sb[:], in_=cmf_d[:])

            XS = [(2 * i, 2) for i in range(8)]
            engs = [nc.gpsimd, nc.scalar]
            xnt_sbs = []
            for ti, (u0, nu) in enumerate(XS):
                t = res_pool.tile([128, 8 * nu * U], sdt, tag=f"xnt{ti}",
                                  name=f"xnt{ti}")
                src = xnt_d[:, u0 * U:(u0 + nu) * U].rearrange(
                    "(k p) c -> p k c", k=8)
                dst = t[:].rearrange("p (k c) -> p k c", k=8)
                engs[ti % 2].dma_start(out=dst, in_=src)
                xnt_sbs.append(t)

            def key_ap(k, u0, ncols):
                # SBUF AP for key columns [u0*U, u0*U+ncols) of d-chunk k
                for (t0, nu), t in zip(XS, xnt_sbs):
                    if t0 <= u0 < t0 + nu:
                        assert u0 + (ncols + U - 1) // U <= t0 + nu, (u0, ncols)
                        off = (u0 - t0) * U
                        return t[:, k * nu * U + off: k * nu * U + off + ncols]
                raise AssertionError(u0)

            ident = res_pool.tile([128, 128], bf16, tag="ident")
            make_identity(nc, ident[:])
            if not trivial_affine:
                gb_sb = res_pool.tile([128, 2 * D], f32, tag="gb")
                g1 = res_pool.tile([1, 2 * D], f32, tag="g1")
                nc.scalar.dma_start(out=g1[:, 0:D], in_=gain_d[:])
                nc.scalar.dma_start(out=g1[:, D:2 * D], in_=bias_d[:])
                nc.vector.partition_broadcast(gb_sb[:], g1[:])

            def emit_p1_stripe(s, h):
                """phase 1 for one stripe: sim matmuls + threshold chain.
                max8 for interior tiles reads PSUM directly so the tau chain
                never waits on the ACT psim->SBUF copies."""
                W = WS[s]
                NJ = W // 2
                qpos = W - 1         # R-position of this slot's query unit
                sim_t = sim_pool.tile([128, 16 * U], f32, tag="sim", bufs=3,
                                      name=f"sim_{s}_{h}")
                m8all = m8_pool.tile([128, 8 * 8], f32, tag="m8all", name=f"m8all_{s}_{h}")
                for jg in range(NJ):
                    psim = psim_pool.tile([128, 512], f32, tag="psim", name=f"psim_{s}_{h}_{jg}")
                    for k in range(8):
                        q_ap = key_ap(k, qpos, U)
                        nc.tensor.matmul(
                            psim[:], q_ap[:, h * 128: h * 128 + 128],
                            key_ap(k, 2 * jg, 512),
                            start=(k == 0), stop=(k == 7))
                    dst = sim_t[:, jg * 512:(jg + 1) * 512]
                    first, last = (jg == 0), (jg == NJ - 1)
                    if first and last:
                        nc.vector.tensor_add(dst, psim[:], cm_sb[:, h * 2 * U:(h + 1) * 2 * U])
                        nc.vector.tensor_add(dst, dst, cmf_sb[:])
                        nc.vector.max(out=m8all[:, jg * 8:(jg + 1) * 8], in_=dst)
                    elif last:
                        nc.vector.tensor_add(dst, psim[:], cm_sb[:, h * 2 * U:(h + 1) * 2 * U])
                        nc.vector.max(out=m8all[:, jg * 8:(jg + 1) * 8], in_=dst)
                    elif first:
                        nc.vector.tensor_add(dst, psim[:], cmf_sb[:])
                        nc.vector.max(out=m8all[:, jg * 8:(jg + 1) * 8], in_=dst)
                    else:
                        nc.vector.max(out=m8all[:, jg * 8:(jg + 1) * 8], in_=psim[:])
                        nc.scalar.copy(dst, psim[:])
                m8f = m8_pool.tile([128, 8], f32, tag="m8f", name=f"m8f_{s}_{h}")
                nc.vector.max(out=m8f[:], in_=m8all[:, 0:NJ * 8])
                tauc = m8_pool.tile([128, 1], f32, tag="tauc", bufs=3, name=f"tauc_{s}_{h}")
                nc.vector.tensor_scalar_max(tauc[:], m8f[:, 7:8], -2.0)
                return sim_t, tauc

            def emit_isge(s, st, jg):
                """A-threshold compares for one jg group (both stripes).
                Emitted 2 groups ahead of their transposes so they stay in
                front of interleaved phase-1 tails in the in-order DVE queue."""
                out = []
                for h in range(2):
                    sim_t, tauc = st[h]
                    a_t = at_pool.tile([128, 512], bf16, tag="at", bufs=4,
                                       name=f"at_{s}_{jg}_{h}")
                    nc.vector.tensor_scalar(
                        a_t[:], sim_t[:, jg * 512:(jg + 1) * 512],
                        tauc[:], None, op0=ALU.is_ge)
                    out.append(a_t)
                return out

            def emit_p23(s, st, pre_at):
                sim_t = [st[0][0], st[1][0]]
                tauc = [st[0][1], st[1][1]]
                W = WS[s]
                NJ = W // 2

                # ---- phase 2: A-build + transpose + fp8 DoubleRow msg matmul ----
                # prefetch this slot's epilogue queries while phase 2 runs
                xqe = [None, None]
                for h in range(2):
                    xqe[h] = xqe_pool.tile([128, D], bf16, tag="xqe", name=f"xqe_{s}_{h}")
                    nc.scalar.dma_start(
                        out=xqe[h][:],
                        in_=xq_d[s * U + h * 128: s * U + (h + 1) * 128, :])

                pmsg = [[pmsg_pool.tile([128, 512], f32, tag="pmsg", name=f"pmsg_{s}_{h}_{dh}")
                         for dh in range(2)] for h in range(2)]
                ats = dict(pre_at)
                for jg in range(NJ):
                    if jg + 2 < NJ and (jg + 2) not in ats:
                        ats[jg + 2] = emit_isge(s, st, jg + 2)
                    atr = [None, None]
                    a_ts = ats.pop(jg)
                    for h in range(2):
                        a_t = a_ts[h]
                        ptr = ptr_pool.tile([128, 512], bf16, tag="ptr", name=f"ptr_{s}_{jg}_{h}")
                        for i in range(4):
                            nc.tensor.transpose(ptr[:, i * 128:(i + 1) * 128],
                                                a_t[:, i * 128:(i + 1) * 128], ident[:])
                        atr[h] = atr_pool.tile([128, 512], fp8, tag="atr", name=f"atr_{s}_{jg}_{h}")
                        nc.scalar.copy(atr[h][:], ptr[:])
                    for i in range(2):
                        jp = jg * 2 + i          # global 256-row key pair
                        xkt = xk_pool.tile([128, 2 * D], fp8, tag="xk", name=f"xk_{s}_{jp}")
                        nc.sync.dma_start(out=xkt[:], in_=xk8_d[jp])
                        xk3 = xkt[:].rearrange("p (two d) -> p two d", two=2)
                        for h in range(2):
                            lhsT = atr[h][:, i * 256:(i + 1) * 256].rearrange(
                                "p (two q) -> p two q", two=2)
                            for dh in range(2):
                                nc.tensor.matmul(
                                    pmsg[h][dh][:], lhsT,
                                    xk3[:, :, dh * 512:(dh + 1) * 512],
                                    start=(jp == 0), stop=(jp == W - 1),
                                    perf_mode=PM.DoubleRow)

                # ---- phase 3: epilogue (xq is host-prescaled; one DVE add) ----
                gelu_af = AF.Identity if _NOGELU else AF.Gelu
                for h in range(2):
                    ot = ot_pool.tile([128, D], bf16, tag="ot", name=f"ot_{s}_{h}")
                    for dh in range(2):
                        pm = pmsg[h][dh]
                        xqs = xqe[h][:, dh * 512:(dh + 1) * 512]
                        if fuse_ok:
                            # xq pre-scaled by mix/msgc
                            nc.vector.tensor_add(pm[:], pm[:], xqs)
                            if trivial_affine:
                                nc.scalar.activation(ot[:, dh * 512:(dh + 1) * 512],
                                                     pm[:], gelu_af, scale=float(msgc))
                                continue
                            nc.vector.tensor_scalar_mul(pm[:], pm[:], float(msgc))
                        else:
                            # xq pre-scaled by mix; scale msg part first
                            nc.vector.tensor_scalar_mul(pm[:], pm[:], float(msgc))
                            nc.vector.tensor_add(pm[:], pm[:], xqs)
                        if not trivial_affine:
                            nc.vector.tensor_mul(pm[:], pm[:], gb_sb[:, dh * 512:(dh + 1) * 512])
                            nc.vector.tensor_add(pm[:], pm[:], gb_sb[:, D + dh * 512: D + (dh + 1) * 512])
                        nc.scalar.activation(ot[:, dh * 512:(dh + 1) * 512], pm[:], gelu_af)
                    nc.scalar.dma_start(
                        out=out_d[s * U + h * 128: s * U + (h + 1) * 128, :],
                        in_=ot[:])

            # software pipeline: P1(s,h0) P1(s,h1) P1(s+1,h0) P23(s) P1(s+1,h1)
            # P1(s+2,h0) P23(s+1) ... so the PE always has a phase-1 block to
            # chew on while the DVE finishes the previous slot's tau chain.
            def emit_pre(s, st):
                # first two jg groups' compares, ahead of the next P1 block
                pre = {0: emit_isge(s, st, 0)}
                if WS[s] // 2 > 1:
                    pre[1] = emit_isge(s, st, 1)
                return pre

            order = list(reversed(range(NSLOT)))
            st = [emit_p1_stripe(order[0], 0), emit_p1_stripe(order[0], 1)]
            pre = emit_pre(order[0], st)
            for si, s in enumerate(order):
                nxt = order[si + 1] if si + 1 < len(order) else None
                nxt_h0 = emit_p1_stripe(nxt, 0) if nxt is not None else None
                emit_p23(s, st, pre)
                if nxt is not None:
                    st = [nxt_h0, emit_p1_stripe(nxt, 1)]
                    pre = emit_pre(nxt, st)
    nc.compile()
    return nc


def _gelu_exact(z):
    from scipy.special import erf
    z64 = z.astype(np.float64)
    return (0.5 * z64 * (1.0 + erf(z64 / np.sqrt(2.0)))).astype(np.float32)


def kernel(x, gain, bias, log_mix, log_scale):
    x = np.asarray(x, dtype=np.float32)
    gain = np.asarray(gain, dtype=np.float32)
    bias = np.asarray(bias, dtype=np.float32)
    mix = float(1.0 / (1.0 + np.exp(-np.float64(log_mix))))
    scale = float(np.log1p(np.exp(np.float64(log_scale))) + 0.01)
    trivial = bool(np.all(gain == 1.0) and np.all(bias == 0.0))

    key = (SIM_DT, trivial, round(mix, 12), round(scale, 12))
    if key not in _PROG_CACHE:
        _PROG_CACHE[key] = _build_program(SIM_DT, trivial, mix, scale)
    nc = _PROG_CACHE[key]

    norms = np.sqrt((x.astype(np.float32) ** 2).sum(-1, keepdims=True)).astype(np.float32)
    xn = x / (norms + np.float32(1e-8))

    in_maps = []
    qunits = []  # per core: list of q unit index per slot
    for c in range(8):
        b, p = c // 2, c % 2
        kus = [WS[s] - 1 - p for s in range(NSLOT)]
        qunits.append(kus)
        xb, xnb = x[b], xn[b]
        if p == 0:
            runits = list(range(16))
        else:
            runits = [0] + list(range(15))
        xnt_np = np.ascontiguousarray(
            np.concatenate([xnb[u * U:(u + 1) * U] for u in runits], 0).T)
        xk8_np = np.ascontiguousarray(
            xb.reshape(T // U, 2, 128, D).transpose(0, 2, 1, 3).reshape(T // U, 128, 2 * D)
        ).astype(ml_dtypes.float8_e4m3)
        if p == 1:
            # odd cores: A columns live in the one-unit-shifted R-space; shift
            # the key stream to match (R-unit 0 is fully masked -> zeros).
            xk8_np = np.concatenate(
                [np.zeros((1, 128, 2 * D), xk8_np.dtype), xk8_np[:T // U - 1]], 0)
        msgc = (1.0 - mix) / K
        pref = np.float32(mix / msgc if msgc > 1e-4 else mix)
        xq_np = np.ascontiguousarray(
            pref * np.concatenate([xb[ku * U:(ku + 1) * U] for ku in kus], 0)
        ).astype(ml_dtypes.bfloat16)
        r = np.arange(128)[:, None]
        f = np.arange(2 * U)[None, :]
        cm = np.zeros((2, 128, 2 * U), dtype=np.float32)
        for h in range(2):
            row = h * 128 + r
            cm[h] = np.where(f <= row + U, 0.0, NEG)
        cmf = np.zeros((128, 2 * U), dtype=np.float32)
        if p == 1:
            cmf[:, :U] = NEG
        m = {"xnt": xnt_np, "xk8": xk8_np, "xq": xq_np, "cmask": cm, "cmaskf": cmf}
        if not trivial:
            m["gain"] = gain.reshape(1, D)
            m["bias"] = bias.reshape(1, D)
        in_maps.append(m)

    global _LAST_IN_MAPS
    _LAST_IN_MAPS = in_maps
    res = run_bass_kernel_spmd(nc, in_maps, list(range(8)), trace=False)

    y = np.empty((B, T, D), dtype=np.float32)
    sc = np.float32(scale)
    for c in range(8):
        b = c // 2
        oc = res.results[c]["out"]
        for s, ku in enumerate(qunits[c]):
            y[b, ku * U:(ku + 1) * U] = oc[s * U:(s + 1) * U].astype(np.float32) * sc

    # exact host fixup for rows with fewer than K neighbors (q < 7)
    for b in range(B):
        nq = K - 1
        msg = np.cumsum(x[b, :nq], axis=0) / np.arange(1, nq + 1, dtype=np.float32)[:, None]
        blended = np.float32(mix) * x[b, :nq] + np.float32(1.0 - mix) * msg
        y[b, :nq] = _gelu_exact(blended * gain + bias) * np.float32(scale)

    return y


# revision 23
# speedup vs baseline: 1.1941x; 1.1941x over previous
"""Trainium2 Bass kernel for causal top-K cosine-similarity GNN message passing.

Module: delta = gelu(mix*x + (1-mix)*msg) * scale, where msg is the mean of
the K=8 causally-preceding neighbors with highest cosine similarity.

Strategy (8 NeuronCores, SPMD):
  - batch b -> core pair (2b, 2b+1). Per batch, 16 query units of 256 rows;
    even core takes units {15,13,...,1}, odd {14,12,...,0}. All cores run an
    identical program under the shared key-width schedule WS=[16,14,...,2]
    (x256 keys); causal masking is data-driven (additive -1e30 mask inputs),
    so padding columns self-mask.
  - resident keys: 16 R-positions of normalized-transposed x. Even cores hold
    units 0..15; odd cores hold [dup(unit0), units 0..14] (one-unit shift), so
    slot s's query unit sits at R-position WS[s]-1 for BOTH parities and the
    sim-matmul stationary (and the diagonal rhs) are SLICED from the resident
    buffer -- no separate query-side DMA stream. The dup/invalid first unit on
    odd cores is masked by an additive first-tile mask (cmf); the last-tile
    diagonal mask (cm) becomes parity-independent (f <= row + U).
  - sim tiles via PE matmul (fp32r operands: full speed, ~2.8e-6 rms noise);
    per-row 8th-largest via the DVE max8 instruction; binary adjacency
    A = (sim >= max(tau, -2)) selects exactly the top-8 (ties measure-zero).
  - msg = (A @ x)/8 as an fp8(e4m3) DoubleRow matmul (2x PE rate, half DMA
    bytes). A is exact in fp8 (0/1); x quantized to e4m3 adds ~8e-3 rel err
    (measured), well inside the 2e-2 gate. A tiles are PE-transposed to put
    the contraction dim on partitions; xk streamed as row-paired fp8 tiles.
  - epilogue: pmsg += (mix/msgc)*x via ACT-copy + DVE add, then a single
    ACT Gelu with input scale=msgc; output stored bf16, final *scale applied
    on the host (monotone rounding unaffected).
  - Rows 0..6 of each batch (fewer than 8 neighbors) are fixed up exactly on
    the host (28 of 16384 rows).
"""

import os
import sys

if "/opt/trn_rl_repo" not in sys.path:
    sys.path.insert(0, "/opt/trn_rl_repo")

_NOGELU = bool(os.environ.get("KERNEL_SIM_NOGELU"))  # CoreSim lacks Gelu

import numpy as np
import ml_dtypes

import concourse.bacc as bacc
import concourse.mybir as mybir
import concourse.tile as tile
from concourse.bass_utils import run_bass_kernel_spmd
from concourse.masks import make_identity

B, T, D, K = 4, 4096, 1024, 8
U = 256                      # unit (query block) size
WS = [16, 14, 12, 10, 8, 6, 4, 2]   # per-slot key width, in units
NSLOT = len(WS)
QPC = NSLOT * U              # query rows per core (2048)
NRES = 16                    # resident R-positions (even: units 0..15; odd: dup+0..14)
NEG = -1.0e30

f32 = mybir.dt.float32
f32r = mybir.dt.float32r
bf16 = mybir.dt.bfloat16
fp8 = mybir.dt.float8e4
AF = mybir.ActivationFunctionType
ALU = mybir.AluOpType
PM = mybir.MatmulPerfMode

SIM_DT = "f32r"              # "f32r" (fast, tf32-like) or "f32" (exact, 4x slower sim)

_PROG_CACHE = {}


def _build_program(sim_dt_key, trivial_affine, mix, scale):
    sdt = f32r if sim_dt_key == "f32r" else f32
    msgc = (1.0 - mix) / K
    # epilogue fuses mix*x into the PSUM accumulator pre-scaled by 1/msgc;
    # requires msgc not absurdly small (harness: mix=0.5 -> msgc=0.0625).
    fuse_ok = msgc > 1e-4

    nc = bacc.Bacc("TRN2", target_bir_lowering=False, debug=False)

    xnt_d = nc.dram_tensor("xnt", [D, NRES * U], sdt, kind="ExternalInput")
    xk8_d = nc.dram_tensor("xk8", [T // U, 128, 2 * D], fp8, kind="ExternalInput")
    # xq is host-prescaled by (mix/msgc) [fuse] or mix [no-fuse] so the
    # epilogue is a single DVE add into PSUM (no ACT copy).
    xq_d = nc.dram_tensor("xq", [QPC, D], bf16, kind="ExternalInput")
    cm_d = nc.dram_tensor("cmask", [2, 128, 2 * U], f32, kind="ExternalInput")
    cmf_d = nc.dram_tensor("cmaskf", [128, 2 * U], f32, kind="ExternalInput")
    if not trivial_affine:
        gain_d = nc.dram_tensor("gain", [1, D], f32, kind="ExternalInput")
        bias_d = nc.dram_tensor("bias", [1, D], f32, kind="ExternalInput")
    out_d = nc.dram_tensor("out", [QPC, D], bf16, kind="ExternalOutput")

    with tile.TileContext(nc) as tc:
        with (
            tc.tile_pool(name="res", bufs=1) as res_pool,
            tc.tile_pool(name="simp", bufs=3) as sim_pool,
            tc.tile_pool(name="xkp", bufs=5) as xk_pool,
            tc.tile_pool(name="atp", bufs=2) as at_pool,
            tc.tile_pool(name="atrp", bufs=4) as atr_pool,
            tc.tile_pool(name="m8p", bufs=2) as m8_pool,
            tc.tile_pool(name="xqep", bufs=2) as xqe_pool,
            tc.tile_pool(name="otp", bufs=2) as ot_pool,
            tc.tile_pool(name="ps_sim", bufs=3, space="PSUM") as psim_pool,
            tc.tile_pool(name="ps_tr", bufs=1, space="PSUM") as ptr_pool,
            tc.tile_pool(name="ps_msg", bufs=4, space="PSUM") as pmsg_pool,
        ):
            # ---- setup: masks first (tiny, needed early), then resident
            # keys (16 R-units) in delivery-ordered tiles.
            cm_sb = res_pool.tile([128, 2 * 2 * U], f32, tag="cm")
            for h in range(2):
                nc.scalar.dma_start(out=cm_sb[:, h * 2 * U:(h + 1) * 2 * U], in_=cm_d[h])
            cmf_sb = res_pool.tile([128, 2 * U], f32, tag="cmf")
            nc.scalar.dma_start(out=cmf_sb[:], in_=cmf_d[:])

            # Per-chunk DMAs: parallelism across the 16 DMA engines comes
            # from many instructions, not one big AP. First group's chunks
            # split across sync+gpsimd (both idle at t=0); the rest stream
            # on gpsimd only, so the consumption-paced xk8 ring on sync
            # never head-of-line-blocks a resident chunk.
            XS = [(0, 2), (2, 2), (4, 2), (6, 2), (8, 4), (12, 4)]
            xnt_sbs = []
            for ti, (u0, nu) in enumerate(XS):
                t = res_pool.tile([128, 8 * nu * U], sdt, tag=f"xnt{ti}",
                                  name=f"xnt{ti}")
                for k in range(8):
                    eng = nc.gpsimd
                    if ti == 0:
                        eng = nc.sync if k % 2 == 0 else nc.gpsimd
                    eng.dma_start(
                        out=t[:, k * nu * U:(k + 1) * nu * U],
                        in_=xnt_d[k * 128:(k + 1) * 128, u0 * U:(u0 + nu) * U])
                xnt_sbs.append(t)

            def key_ap(k, u0, ncols):
                # SBUF AP for key columns [u0*U, u0*U+ncols) of d-chunk k
                for (t0, nu), t in zip(XS, xnt_sbs):
                    if t0 <= u0 < t0 + nu:
                        assert u0 + (ncols + U - 1) // U <= t0 + nu, (u0, ncols)
                        off = (u0 - t0) * U
                        return t[:, k * nu * U + off: k * nu * U + off + ncols]
                raise AssertionError(u0)

            ident = res_pool.tile([128, 128], bf16, tag="ident")
            make_identity(nc, ident[:])
            if not trivial_affine:
                gb_sb = res_pool.tile([128, 2 * D], f32, tag="gb")
                g1 = res_pool.tile([1, 2 * D], f32, tag="g1")
                nc.scalar.dma_start(out=g1[:, 0:D], in_=gain_d[:])
                nc.scalar.dma_start(out=g1[:, D:2 * D], in_=bias_d[:])
                nc.vector.partition_broadcast(gb_sb[:], g1[:])

            def emit_p1_stripe(s, h):
                """phase 1 for one stripe: sim matmuls + threshold chain.
                max8 for interior tiles reads PSUM directly so the tau chain
                never waits on the ACT psim->SBUF copies."""
                W = WS[s]
                NJ = W // 2
                qpos = W - 1         # R-position of this slot's query unit
                sim_t = sim_pool.tile([128, 16 * U], f32, tag="sim", bufs=3,
                                      name=f"sim_{s}_{h}")
                m8all = m8_pool.tile([128, 8 * 8], f32, tag="m8all", name=f"m8all_{s}_{h}")
                for jg in range(NJ):
                    psim = psim_pool.tile([128, 512], f32, tag="psim", name=f"psim_{s}_{h}_{jg}")
                    for k in range(8):
                        q_ap = key_ap(k, qpos, U)
                        nc.tensor.matmul(
                            psim[:], q_ap[:, h * 128: h * 128 + 128],
                            key_ap(k, 2 * jg, 512),
                            start=(k == 0), stop=(k == 7))
                    dst = sim_t[:, jg * 512:(jg + 1) * 512]
                    first, last = (jg == 0), (jg == NJ - 1)
                    if first and last:
                        nc.vector.tensor_add(dst, psim[:], cm_sb[:, h * 2 * U:(h + 1) * 2 * U])
                        nc.vector.tensor_add(dst, dst, cmf_sb[:])
                        nc.vector.max(out=m8all[:, jg * 8:(jg + 1) * 8], in_=dst)
                    elif last:
                        nc.vector.tensor_add(dst, psim[:], cm_sb[:, h * 2 * U:(h + 1) * 2 * U])
                        nc.vector.max(out=m8all[:, jg * 8:(jg + 1) * 8], in_=dst)
                    elif first:
                        nc.vector.tensor_add(dst, psim[:], cmf_sb[:])
                        nc.vector.max(out=m8all[:, jg * 8:(jg + 1) * 8], in_=dst)
                    else:
                        nc.vector.max(out=m8all[:, jg * 8:(jg + 1) * 8], in_=psim[:])
                        nc.scalar.copy(dst, psim[:])
                m8f = m8_pool.tile([128, 8], f32, tag="m8f", name=f"m8f_{s}_{h}")
                nc.vector.max(out=m8f[:], in_=m8all[:, 0:NJ * 8])
                tauc = m8_pool.tile([128, 1], f32, tag="tauc", bufs=3, name=f"tauc_{s}_{h}")
                nc.vector.tensor_scalar_max(tauc[:], m8f[:, 7:8], -2.0)
                return sim_t, tauc

            def emit_isge(s, st, jg):
                """A-threshold compares for one jg group (both stripes).
                Emitted 2 groups ahead of their transposes so they stay in
                front of interleaved phase-1 tails in the in-order DVE queue."""
                out = []
                for h in range(2):
                    sim_t, tauc = st[h]
                    a_t = at_pool.tile([128, 512], bf16, tag="at", bufs=4,
                                       name=f"at_{s}_{jg}_{h}")
                    nc.vector.tensor_scalar(
                        a_t[:], sim_t[:, jg * 512:(jg + 1) * 512],
                        tauc[:], None, op0=ALU.is_ge)
                    out.append(a_t)
                return out

            def emit_p23(s, st, pre_at):
                sim_t = [st[0][0], st[1][0]]
                tauc = [st[0][1], st[1][1]]
                W = WS[s]
                NJ = W // 2

                # ---- phase 2: A-build + transpose + fp8 DoubleRow msg matmul ----
                # prefetch this slot's epilogue queries while phase 2 runs
                xqe = [None, None]
                for h in range(2):
                    xqe[h] = xqe_pool.tile([128, D], bf16, tag="xqe", name=f"xqe_{s}_{h}")
                    nc.scalar.dma_start(
                        out=xqe[h][:],
                        in_=xq_d[s * U + h * 128: s * U + (h + 1) * 128, :])

                pmsg = [[pmsg_pool.tile([128, 512], f32, tag="pmsg", name=f"pmsg_{s}_{h}_{dh}")
                         for dh in range(2)] for h in range(2)]
                ats = dict(pre_at)
                for jg in range(NJ):
                    if jg + 2 < NJ and (jg + 2) not in ats:
                        ats[jg + 2] = emit_isge(s, st, jg + 2)
                    atr = [None, None]
                    a_ts = ats.pop(jg)
                    for h in range(2):
                        a_t = a_ts[h]
                        ptr = ptr_pool.tile([128, 512], bf16, tag="ptr", name=f"ptr_{s}_{jg}_{h}")
                        for i in range(4):
                            nc.tensor.transpose(ptr[:, i * 128:(i + 1) * 128],
                                                a_t[:, i * 128:(i + 1) * 128], ident[:])
                        atr[h] = atr_pool.tile([128, 512], fp8, tag="atr", name=f"atr_{s}_{jg}_{h}")
                        nc.scalar.copy(atr[h][:], ptr[:])
                    for i in range(2):
                        jp = jg * 2 + i          # global 256-row key pair
                        xkt = xk_pool.tile([128, 2 * D], fp8, tag="xk", name=f"xk_{s}_{jp}")
                        nc.sync.dma_start(out=xkt[:], in_=xk8_d[jp])
                        xk3 = xkt[:].rearrange("p (two d) -> p two d", two=2)
                        for h in range(2):
                            lhsT = atr[h][:, i * 256:(i + 1) * 256].rearrange(
                                "p (two q) -> p two q", two=2)
                            for dh in range(2):
                                nc.tensor.matmul(
                                    pmsg[h][dh][:], lhsT,
                                    xk3[:, :, dh * 512:(dh + 1) * 512],
                                    start=(jp == 0), stop=(jp == W - 1),
                                    perf_mode=PM.DoubleRow)

                # ---- phase 3: epilogue (xq is host-prescaled; one DVE add) ----
                gelu_af = AF.Identity if _NOGELU else AF.Gelu
                for h in range(2):
                    ot = ot_pool.tile([128, D], bf16, tag="ot", name=f"ot_{s}_{h}")
                    for dh in range(2):
                        pm = pmsg[h][dh]
                        xqs = xqe[h][:, dh * 512:(dh + 1) * 512]
                        if fuse_ok:
                            # xq pre-scaled by mix/msgc
                            nc.vector.tensor_add(pm[:], pm[:], xqs)
                            if trivial_affine:
                                nc.scalar.activation(ot[:, dh * 512:(dh + 1) * 512],
                                                     pm[:], gelu_af, scale=float(msgc))
                                continue
                            nc.vector.tensor_scalar_mul(pm[:], pm[:], float(msgc))
                        else:
                            # xq pre-scaled by mix; scale msg part first
                            nc.vector.tensor_scalar_mul(pm[:], pm[:], float(msgc))
                            nc.vector.tensor_add(pm[:], pm[:], xqs)
                        if not trivial_affine:
                            nc.vector.tensor_mul(pm[:], pm[:], gb_sb[:, dh * 512:(dh + 1) * 512])
                            nc.vector.tensor_add(pm[:], pm[:], gb_sb[:, D + dh * 512: D + (dh + 1) * 512])
                        nc.scalar.activation(ot[:, dh * 512:(dh + 1) * 512], pm[:], gelu_af)
                    nc.scalar.dma_start(
                        out=out_d[s * U + h * 128: s * U + (h + 1) * 128, :],
                        in_=ot[:])

            # software pipeline: P1(s,h0) P1(s,h1) P1(s+1,h0) P23(s) P1(s+1,h1)
            # P1(s+2,h0) P23(s+1) ... so the PE always has a phase-1 block to
            # chew on while the DVE finishes the previous slot's tau chain.
            def emit_pre(s, st):
                # first two jg groups' compares, ahead of the next P1 block
                pre = {0: emit_isge(s, st, 0)}
                if WS[s] // 2 > 1:
                    pre[1] = emit_isge(s, st, 1)
                return pre

            order = list(reversed(range(NSLOT)))
            st = [emit_p1_stripe(order[0], 0), emit_p1_stripe(order[0], 1)]
            pre = emit_pre(order[0], st)
            for si, s in enumerate(order):
                nxt = order[si + 1] if si + 1 < len(order) else None
                nxt_h0 = emit_p1_stripe(nxt, 0) if nxt is not None else None
                emit_p23(s, st, pre)
                if nxt is not None:
                    st = [nxt_h0, emit_p1_stripe(nxt, 1)]
                    pre = emit_pre(nxt, st)
    nc.compile()
    return nc


def _gelu_exact(z):
    from scipy.special import erf
    z64 = z.astype(np.float64)
    return (0.5 * z64 * (1.0 + erf(z64 / np.sqrt(2.0)))).astype(np.float32)


def kernel(x, gain, bias, log_mix, log_scale):
    x = np.asarray(x, dtype=np.float32)
    gain = np.asarray(gain, dtype=np.float32)
    bias = np.asarray(bias, dtype=np.float32)
    mix = float(1.0 / (1.0 + np.exp(-np.float64(log_mix))))
    scale = float(np.log1p(np.exp(np.float64(log_scale))) + 0.01)
    trivial = bool(np.all(gain == 1.0) and np.all(bias == 0.0))

    key = (SIM_DT, trivial, round(mix, 12), round(scale, 12))
    if key not in _PROG_CACHE:
        _PROG_CACHE[key] = _build_program(SIM_DT, trivial, mix, scale)
    nc = _PROG_CACHE[key]

    norms = np.sqrt((x.astype(np.float32) ** 2).sum(-1, keepdims=True)).astype(np.float32)
    xn = x / (norms + np.float32(1e-8))

    in_maps = []
    qunits = []  # per core: list of q unit index per slot
    for c in range(8):
        b, p = c // 2, c % 2
        kus = [WS[s] - 1 - p for s in range(NSLOT)]
        qunits.append(kus)
        xb, xnb = x[b], xn[b]
        if p == 0:
            runits = list(range(16))
        else:
            runits = [0] + list(range(15))
        xnt_np = np.ascontiguousarray(
            np.concatenate([xnb[u * U:(u + 1) * U] for u in runits], 0).T)
        xk8_np = np.ascontiguousarray(
            xb.reshape(T // U, 2, 128, D).transpose(0, 2, 1, 3).reshape(T // U, 128, 2 * D)
        ).astype(ml_dtypes.float8_e4m3)
        if p == 1:
            # odd cores: A columns live in the one-unit-shifted R-space; shift
            # the key stream to match (R-unit 0 is fully masked -> zeros).
            xk8_np = np.concatenate(
                [np.zeros((1, 128, 2 * D), xk8_np.dtype), xk8_np[:T // U - 1]], 0)
        msgc = (1.0 - mix) / K
        pref = np.float32(mix / msgc if msgc > 1e-4 else mix)
        xq_np = np.ascontiguousarray(
            pref * np.concatenate([xb[ku * U:(ku + 1) * U] for ku in kus], 0)
        ).astype(ml_dtypes.bfloat16)
        r = np.arange(128)[:, None]
        f = np.arange(2 * U)[None, :]
        cm = np.zeros((2, 128, 2 * U), dtype=np.float32)
        for h in range(2):
            row = h * 128 + r
            cm[h] = np.where(f <= row + U, 0.0, NEG)
        cmf = np.zeros((128, 2 * U), dtype=np.float32)
        if p == 1:
            cmf[:, :U] = NEG
        m = {"xnt": xnt_np, "xk8": xk8_np, "xq": xq_np, "cmask": cm, "cmaskf": cmf}
        if not trivial:
            m["gain"] = gain.reshape(1, D)
            m["bias"] = bias.reshape(1, D)
        in_maps.append(m)

    global _LAST_IN_MAPS
    _LAST_IN_MAPS = in_maps
    res = run_bass_kernel_spmd(nc, in_maps, list(range(8)), trace=False)

    y = np.empty((B, T, D), dtype=np.float32)
    sc = np.float32(scale)
    for c in range(8):
        b = c // 2
        oc = res.results[c]["out"]
        for s, ku in enumerate(qunits[c]):
            y[b, ku * U:(ku + 1) * U] = oc[s * U:(s + 1) * U].astype(np.float32) * sc

    # exact host fixup for rows with fewer than K neighbors (q < 7)
    for b in range(B):
        nq = K - 1
        msg = np.cumsum(x[b, :nq], axis=0) / np.arange(1, nq + 1, dtype=np.float32)[:, None]
        blended = np.float32(mix) * x[b, :nq] + np.float32(1.0 - mix) * msg
        y[b, :nq] = _gelu_exact(blended * gain + bias) * np.float32(scale)

    return y
